# revision 1
# baseline (speedup 1.0000x reference)
"""Trainium2 Bass kernel for the CustomRNNDecoder (GRU decoder) problem.

Data-parallel over batch: 8 NeuronCores x 8 batch rows each. Everything on
device is kept "D-major" (hidden/gate dim on partitions, batch on the free
dim) so the sequential GRU scan needs no transposes:

  - gh.T[gate_chunk, b] = sum_k w_hh.T-tile[k, gate_chunk].T @ h.T[k, b]
    (stationary = weight tile [128, 128] bf16, moving = h [128, 8] bf16)
  - gate math on Vector/Scalar engines on [128, 4, 8] tiles (free dim 32)
  - input-side projection Gi = x @ w_ih.T + biases precomputed in bulk
  - output projection lin_w @ h.T done in bulk after the scan

Host side does only the embedding gather + layout shuffles (pure memcpy-type
work); all FLOPs run on the NeuronCores.
"""

import os
import sys

import numpy as np

sys.path.insert(0, "/opt/trn_rl_repo")

import ml_dtypes

BF16 = ml_dtypes.bfloat16

# Problem constants (hardcoded per the harness contract).
B, U, V, D, J = 64, 256, 32000, 512, 640
T = U + 1            # 257 scan steps
NCORES = 8
BL = B // NCORES     # 8 batch rows per core
KD = D // 128        # 4 contraction chunks
G3 = (3 * D) // 128  # 12 gate chunks
JC = J // 128        # 5 output chunks
NTOK = T * BL        # 2056 (token, batch) pairs per core
BOS = 0

# Token blocks for bulk matmuls: blocks of 64 time steps (= 512 free cols).
TBLK = 64

_PROG_CACHE = {}


def _build_program(t_steps, repeat=1, repeat_phases=("gi", "scan", "out")):
    import concourse.bass as bass
    import concourse.tile as tile
    from concourse import bacc, mybir

    f32 = mybir.dt.float32
    bf16 = mybir.dt.bfloat16
    ds = bass.ds
    AF = mybir.ActivationFunctionType

    ntok = t_steps * BL
    tblocks = [
        (i * TBLK, min(TBLK, t_steps - i * TBLK))
        for i in range((t_steps + TBLK - 1) // TBLK)
    ]

    nc = bacc.Bacc(
        "TRN2",
        target_bir_lowering=False,
        debug=False,
        enable_asserts=True,
        num_devices=1,
    )

    xt_d = nc.dram_tensor("xt", [128, KD, ntok], bf16, kind="ExternalInput").ap()
    wih_d = nc.dram_tensor("wih", [128, KD, G3, 128], bf16, kind="ExternalInput").ap()
    whh_d = nc.dram_tensor("whh", [128, KD, G3, 128], bf16, kind="ExternalInput").ap()
    lin_d = nc.dram_tensor("lin", [128, KD, JC, 128], bf16, kind="ExternalInput").ap()
    brz_d = nc.dram_tensor("brz", [128, 8], f32, kind="ExternalInput").ap()
    bni_d = nc.dram_tensor("bni", [128, KD], f32, kind="ExternalInput").ap()
    bhn_d = nc.dram_tensor("bhn", [128, KD, BL], f32, kind="ExternalInput").ap()
    linb_d = nc.dram_tensor("linb", [128, JC], f32, kind="ExternalInput").ap()
    h0_d = nc.dram_tensor("h0", [128, KD, BL], bf16, kind="ExternalInput").ap()
    outp_d = nc.dram_tensor("outp", [128, JC, ntok], f32, kind="ExternalOutput").ap()

    with tile.TileContext(nc) as tc:
        with tc.tile_pool(name="const", bufs=1) as constp:
            # Load everything resident into SBUF.
            xt = constp.tile([128, KD, ntok], bf16)
            nc.sync.dma_start(xt[:], xt_d[:])
            wih = constp.tile([128, KD, G3, 128], bf16)
            nc.sync.dma_start(wih[:], wih_d[:])
            whh = constp.tile([128, KD, G3, 128], bf16)
            nc.sync.dma_start(whh[:], whh_d[:])
            lin = constp.tile([128, KD, JC, 128], bf16)
            nc.sync.dma_start(lin[:], lin_d[:])
            brz = constp.tile([128, 8], f32)
            nc.sync.dma_start(brz[:], brz_d[:])
            bni = constp.tile([128, KD], f32)
            nc.sync.dma_start(bni[:], bni_d[:])
            bhn = constp.tile([128, KD, BL], f32)
            nc.sync.dma_start(bhn[:], bhn_d[:])
            linb = constp.tile([128, JC], f32)
            nc.sync.dma_start(linb[:], linb_d[:])
            h0 = constp.tile([128, KD, BL], bf16)
            nc.sync.dma_start(h0[:], h0_d[:])

            # Big persistent buffers.
            git = constp.tile([128, G3, ntok], bf16)          # input-side gates
            hall = constp.tile([128, KD, t_steps, BL], bf16)  # hidden history

            def emit_phase1(gips):
                # Gi = x @ w_ih.T (+ biases), bulk, token-blocked.
                for g in range(G3):
                    bias_ap = brz[:, g : g + 1] if g < 8 else bni[:, g - 8 : g - 7]
                    for t0, tn in tblocks:
                        sz = tn * BL
                        ps = gips.tile([128, TBLK * BL], f32, tag="gi")
                        for k in range(KD):
                            nc.tensor.matmul(
                                ps[:, :sz],
                                wih[:, k, g, :],
                                xt[:, k, ds(t0 * BL, sz)],
                                start=(k == 0),
                                stop=(k == KD - 1),
                            )
                        nc.vector.tensor_scalar(
                            git[:, g, ds(t0 * BL, sz)],
                            ps[:, :sz],
                            bias_ap,
                            None,
                            mybir.AluOpType.add,
                        )

            def emit_scan_pe_only(t, scanps, sinkp):
                # PE throughput probe: the 48 LDW+MM pairs of one step with a
                # constant h (no cross-step dependency, no elementwise).
                ps_r = scanps.tile([128, KD, BL], f32, tag="ps_r")
                ps_z = scanps.tile([128, KD, BL], f32, tag="ps_z")
                ps_n = scanps.tile([128, KD, BL], f32, tag="ps_n")
                for gg, ps in ((0, ps_r), (2, ps_n), (1, ps_z)):
                    for j in range(KD):
                        g = gg * KD + j
                        for k in range(KD):
                            nc.tensor.matmul(
                                ps[:, j, :],
                                whh[:, k, g, :],
                                h0[:, k, :],
                                start=(k == 0),
                                stop=(k == KD - 1),
                            )
                # minimal evacuation so psum slots recycle (ACT, off PE path)
                for nm, ps in (("skr", ps_r), ("skz", ps_z), ("skn", ps_n)):
                    sink = sinkp.tile([128, KD, BL], f32, tag=nm)
                    nc.scalar.copy(sink[:], ps[:])

            def emit_scan_ldw_reuse(t, scanps, sinkp):
                # Probe: same stationary for the 4 k-matmuls of each group
                # (mathematically wrong; tests walrus LDW dedupe + LDW cost).
                ps_r = scanps.tile([128, KD, BL], f32, tag="ps_r")
                ps_z = scanps.tile([128, KD, BL], f32, tag="ps_z")
                ps_n = scanps.tile([128, KD, BL], f32, tag="ps_n")
                for gg, ps in ((0, ps_r), (2, ps_n), (1, ps_z)):
                    for j in range(KD):
                        g = gg * KD + j
                        for k in range(KD):
                            nc.tensor.matmul(
                                ps[:, j, :],
                                whh[:, 0, g, :],
                                h0[:, k, :],
                                start=(k == 0),
                                stop=(k == KD - 1),
                            )
                for nm, ps in (("skr", ps_r), ("skz", ps_z), ("skn", ps_n)):
                    sink = sinkp.tile([128, KD, BL], f32, tag=nm)
                    nc.scalar.copy(sink[:], ps[:])

            def emit_scan_bm(t, scanps, sinkp):
                # Probe: batch-major form — stationary h.T [128,8], moving
                # w_hh.T 512-col chunks. psum [8, 512] x3.
                pss = []
                for i in range(3):
                    ps_bm = scanps.tile([8, 512], f32, tag=f"bm{i}", name=f"bm{i}")
                    pss.append(ps_bm)
                for i, ps in enumerate(pss):
                    for k in range(KD):
                        nc.tensor.matmul(
                            ps[:],
                            h0[:, k, :],
                            whh[:, k, 4 * i : 4 * (i + 1), :],
                            start=(k == 0),
                            stop=(k == KD - 1),
                        )
                for i, ps in enumerate(pss):
                    sink = sinkp.tile([8, 512], f32, tag=f"sbm{i}", name=f"sbm{i}")
                    nc.scalar.copy(sink[:], ps[:])

            def emit_scan_step(t, scanps, ew, feedback=True):
                h_prev = (h0 if t == 0 else hall[:, :, t - 1, :]) if feedback else h0
                ps_r = scanps.tile([128, KD, BL], f32, tag="ps_r")
                ps_z = scanps.tile([128, KD, BL], f32, tag="ps_z")
                ps_n = scanps.tile([128, KD, BL], f32, tag="ps_n")
                # Matmul order r, n, z: the long n-gate elementwise chain
                # (t1a..tanh..d) overlaps the z-group matmuls, leaving only
                # the short z tail (pre_z' -> zc -> e -> h') on the critical
                # path before the next step can start.
                for gg, ps in ((0, ps_r), (2, ps_n), (1, ps_z)):
                    for j in range(KD):
                        g = gg * KD + j
                        for k in range(KD):
                            nc.tensor.matmul(
                                ps[:, j, :],
                                whh[:, k, g, :],
                                h_prev[:, k, :],
                                start=(k == 0),
                                stop=(k == KD - 1),
                            )
                # Gate math (fp32 intermediates; h stored bf16).
                gi_r = git[:, 0:KD, ds(t * BL, BL)]
                gi_z = git[:, KD : 2 * KD, ds(t * BL, BL)]
                gi_n = git[:, 2 * KD : 3 * KD, ds(t * BL, BL)]

                pre_r = ew.tile([128, KD, BL], f32, tag="pre_r")
                nc.vector.tensor_add(pre_r[:], ps_r[:], gi_r)
                r = ew.tile([128, KD, BL], f32, tag="r")
                nc.scalar.activation(r[:], pre_r[:], AF.Sigmoid)

                t1a = ew.tile([128, KD, BL], f32, tag="t1a")
                nc.vector.tensor_add(t1a[:], ps_n[:], bhn[:])
                t1 = ew.tile([128, KD, BL], f32, tag="t1")
                nc.vector.tensor_mul(t1[:], t1a[:], r[:])
                t2 = ew.tile([128, KD, BL], f32, tag="t2")
                nc.vector.tensor_add(t2[:], t1[:], gi_n)
                n_g = ew.tile([128, KD, BL], f32, tag="n_g")
                nc.scalar.activation(n_g[:], t2[:], AF.Tanh)
                d_g = ew.tile([128, KD, BL], f32, tag="d_g")
                nc.vector.tensor_sub(d_g[:], n_g[:], h_prev[:])

                # pre_z' = -(ps_z + gi_z) fused in one op; zc = sigmoid(pre_z')
                pre_z = ew.tile([128, KD, BL], f32, tag="pre_z")
                nc.vector.scalar_tensor_tensor(
                    pre_z[:], ps_z[:], -1.0, gi_z,
                    mybir.AluOpType.mult, mybir.AluOpType.subtract,
                )
                zc = ew.tile([128, KD, BL], f32, tag="zc")
                nc.scalar.activation(zc[:], pre_z[:], AF.Sigmoid)

                e_g = ew.tile([128, KD, BL], f32, tag="e_g")
                nc.vector.tensor_mul(e_g[:], zc[:], d_g[:])
                nc.vector.tensor_add(hall[:, :, t, :], h_prev[:], e_g[:])

            def emit_phase3(ops, oevac):
                # out = h @ lin_w.T + lin_b, bulk, then DMA out.
                for c in range(JC):
                    for t0, tn in tblocks:
                        sz = tn * BL
                        ps = ops.tile([128, TBLK * BL], f32, tag="op")
                        for k in range(KD):
                            nc.tensor.matmul(
                                ps[:, :sz],
                                lin[:, k, c, :],
                                hall[:, k, ds(t0, tn), :],
                                start=(k == 0),
                                stop=(k == KD - 1),
                            )
                        ot = oevac.tile([128, TBLK * BL], f32, tag="ot")
                        nc.vector.tensor_scalar(
                            ot[:, :sz],
                            ps[:, :sz],
                            linb[:, c : c + 1],
                            None,
                            mybir.AluOpType.add,
                        )
                        nc.sync.dma_start(outp_d[:, c, ds(t0 * BL, sz)], ot[:, :sz])

            # repeat > 1 re-runs phases listed in repeat_phases (used only
            # for wall-clock timing via deltas; the output is idempotent).
            def n_reps(phase):
                return repeat if phase in repeat_phases else 1

            for _rep in range(n_reps("gi")):
                with tc.tile_pool(name="gips", bufs=2, space="PSUM") as gips:
                    emit_phase1(gips)
            if "scanpe" in repeat_phases:
                for _rep in range(repeat):
                    with tc.tile_pool(name="scanps", bufs=2, space="PSUM") as sps, \
                         tc.tile_pool(name="sink", bufs=2) as sinkp:
                        for t in range(t_steps):
                            emit_scan_pe_only(t, sps, sinkp)
            for probe_name, probe_fn in (
                ("ldwreuse", emit_scan_ldw_reuse),
                ("bm", emit_scan_bm),
            ):
                if probe_name in repeat_phases:
                    for _rep in range(repeat):
                        with tc.tile_pool(name="scanps", bufs=2, space="PSUM") as sps, \
                             tc.tile_pool(name="sink", bufs=2) as sinkp:
                            for t in range(t_steps):
                                probe_fn(t, sps, sinkp)
            if "scannofb" in repeat_phases:
                for _rep in range(repeat):
                    with tc.tile_pool(name="scanps", bufs=2, space="PSUM") as sps, \
                         tc.tile_pool(name="ew", bufs=3) as ewp:
                        for t in range(t_steps):
                            emit_scan_step(t, sps, ewp, feedback=False)
            for _rep in range(n_reps("scan")):
                with tc.tile_pool(name="scanps", bufs=2, space="PSUM") as scanps, \
                     tc.tile_pool(name="ew", bufs=3) as ew:
                    for t in range(t_steps):
                        emit_scan_step(t, scanps, ew)
            for _rep in range(n_reps("out")):
                with tc.tile_pool(name="ops", bufs=2, space="PSUM") as ops, \
                     tc.tile_pool(name="oevac", bufs=3) as oevac:
                    emit_phase3(ops, oevac)

    nc.compile()
    return nc


def _get_program(t_steps, repeat=1, repeat_phases=("gi", "scan", "out")):
    key = (t_steps, repeat, tuple(repeat_phases))
    if key not in _PROG_CACHE:
        _PROG_CACHE[key] = _build_program(t_steps, repeat, repeat_phases)
    return _PROG_CACHE[key]


def kernel(src_tokens, src_lengths, embed_w, w_ih, w_hh, b_ih, b_hh, lin_w, lin_b,
           init_state, _t_steps=T, _want_results=False, _trace=False, _tmpdir=None,
           _repeat=1, _repeat_phases=("gi", "scan", "out"), **_ignored):
    from concourse.bass_utils import run_bass_kernel_spmd

    src_tokens = np.asarray(src_tokens)
    embed_w = np.asarray(embed_w, dtype=np.float32)
    w_ih = np.asarray(w_ih, dtype=np.float32)
    w_hh = np.asarray(w_hh, dtype=np.float32)
    b_ih = np.asarray(b_ih, dtype=np.float32)
    b_hh = np.asarray(b_hh, dtype=np.float32)
    lin_w = np.asarray(lin_w, dtype=np.float32)
    lin_b = np.asarray(lin_b, dtype=np.float32)
    init_state = np.asarray(init_state, dtype=np.float32)

    t_steps = _t_steps
    ntok = t_steps * BL

    # Host prep: embedding gather + layout shuffles (no FLOPs).
    tokens = np.concatenate(
        [np.full((B, 1), BOS, dtype=src_tokens.dtype), src_tokens], axis=1
    )[:, :t_steps]                                   # [B, T]
    X = embed_w[tokens].astype(BF16)                 # [B, T, D]

    def dmaj(vec):  # [D] -> [128, KD]
        return np.ascontiguousarray(vec.reshape(KD, 128).T)

    wih_t = np.ascontiguousarray(
        w_ih.reshape(G3, 128, KD, 128).transpose(3, 2, 0, 1)).astype(BF16)
    whh_t = np.ascontiguousarray(
        w_hh.reshape(G3, 128, KD, 128).transpose(3, 2, 0, 1)).astype(BF16)
    lin_t = np.ascontiguousarray(
        lin_w.reshape(JC, 128, KD, 128).transpose(3, 2, 0, 1)).astype(BF16)
    brz = np.ascontiguousarray((b_ih + b_hh)[: 2 * D].reshape(8, 128).T)
    bni = dmaj(b_ih[2 * D :])
    bhn = np.ascontiguousarray(
        np.broadcast_to(dmaj(b_hh[2 * D :])[:, :, None], (128, KD, BL)))
    linb = np.ascontiguousarray(lin_b.reshape(JC, 128).T)
    h0 = np.ascontiguousarray(
        np.broadcast_to(dmaj(init_state)[:, :, None], (128, KD, BL))).astype(BF16)

    shared = {
        "wih": wih_t, "whh": whh_t, "lin": lin_t,
        "brz": brz.astype(np.float32), "bni": bni.astype(np.float32),
        "bhn": bhn.astype(np.float32), "linb": linb.astype(np.float32),
        "h0": h0,
    }
    in_maps = []
    for c in range(NCORES):
        xc = X[c * BL : (c + 1) * BL]                # [BL, T, D]
        xt = np.ascontiguousarray(
            xc.reshape(BL, t_steps, KD, 128).transpose(3, 2, 1, 0)
        ).reshape(128, KD, ntok)
        in_maps.append({**shared, "xt": np.ascontiguousarray(xt)})

    nc = _get_program(t_steps, _repeat, _repeat_phases)
    res = run_bass_kernel_spmd(
        nc, in_maps, core_ids=list(range(NCORES)), trace=_trace, tmpdir=_tmpdir
    )

    out = np.empty((B, t_steps, J), dtype=np.float32)
    for c in range(NCORES):
        o = res.results[c]["outp"]                   # [128, JC, ntok]
        o = o.reshape(128, JC, t_steps, BL).transpose(3, 2, 1, 0)  # [BL,T,JC,128]
        out[c * BL : (c + 1) * BL] = o.reshape(BL, t_steps, J)
    if _want_results:
        return out, res
    return out


if __name__ == "__main__":
    # Quick smoke test with a tiny number of steps.
    t_steps = int(os.environ.get("KERNEL_T", "8"))
    rng = np.random.default_rng(0)
    ins = {
        "src_tokens": rng.integers(0, V, size=(B, U)).astype(np.int64),
        "src_lengths": rng.integers(1, U, size=(B,)).astype(np.int32),
        "embed_w": (rng.standard_normal((V, D)) * 0.02).astype(np.float32),
        "w_ih": (rng.standard_normal((3 * D, D)) / np.sqrt(D)).astype(np.float32),
        "w_hh": (rng.standard_normal((3 * D, D)) / np.sqrt(D)).astype(np.float32),
        "b_ih": (rng.standard_normal(3 * D) * 0.01).astype(np.float32),
        "b_hh": (rng.standard_normal(3 * D) * 0.01).astype(np.float32),
        "lin_w": (rng.standard_normal((J, D)) / np.sqrt(D)).astype(np.float32),
        "lin_b": (rng.standard_normal(J) * 0.01).astype(np.float32),
        "init_state": rng.standard_normal(D).astype(np.float32),
    }
    actual = kernel(**ins, _t_steps=t_steps)

    # numpy reference for t_steps
    tokens = np.concatenate(
        [np.zeros((B, 1), dtype=np.int64), ins["src_tokens"]], axis=1)[:, :t_steps]
    x_all = ins["embed_w"][tokens]
    h = np.broadcast_to(ins["init_state"], (B, D)).astype(np.float32)
    outs = []
    for t in range(t_steps):
        gi = x_all[:, t] @ ins["w_ih"].T + ins["b_ih"]
        gh = h @ ins["w_hh"].T + ins["b_hh"]
        i_r, i_z, i_n = np.split(gi, 3, axis=-1)
        h_r, h_z, h_n = np.split(gh, 3, axis=-1)
        r = 1 / (1 + np.exp(-(i_r + h_r)))
        z = 1 / (1 + np.exp(-(i_z + h_z)))
        n = np.tanh(i_n + r * h_n)
        h = (1 - z) * n + z * h
        outs.append(h @ ins["lin_w"].T + ins["lin_b"])
    expected = np.stack(outs, axis=1)
    err = np.abs(actual - expected)
    rel = np.linalg.norm(actual - expected) / np.linalg.norm(expected)
    print("max abs err:", err.max(), "rel l2:", rel)



# revision 23
# speedup vs baseline: 1559.1543x; 1559.1543x over previous
"""Trainium2 Bass kernel for the CustomRNNDecoder (GRU decoder) problem.

Data-parallel over batch: 8 NeuronCores x 8 batch rows each. Everything on
device is kept "D-major" (hidden/gate dim on partitions, batch on the free
dim) so the sequential GRU scan needs no transposes:

  - gh.T[gate_chunk, b] = sum_k w_hh.T-tile[k, gate_chunk].T @ h.T[k, b]
    (stationary = weight tile [128, 128] bf16, moving = h [128, 8] bf16)
  - gate math on Vector/Scalar engines on [128, 4, 8] tiles (free dim 32)
  - input-side projection Gi = x @ w_ih.T + biases precomputed in bulk
  - output projection lin_w @ h.T done in bulk after the scan

The whole compute body (gi, scan, out) is wrapped in a hardware For_i loop
whose trip count comes from an ExternalInput scalar ("nrep"), so device-side
repeats for timing need no recompilation and no program-size growth.  The
jitted PJRT executable is cached module-level, so repeated kernel() calls
pay only data transfer + device execution.

Host side does only the embedding gather + layout shuffles (pure memcpy-type
work); all FLOPs run on the NeuronCores.
"""

import os
import sys

import numpy as np

sys.path.insert(0, "/opt/trn_rl_repo")

import ml_dtypes

BF16 = np.float16  # fp16: better mantissa than bf16 at these ranges, same matmul speed

# Problem constants (hardcoded per the harness contract).
B, U, V, D, J = 64, 256, 32000, 512, 640
T = U + 1            # 257 scan steps
NCORES = 8
BL = B // NCORES     # 8 batch rows per core
KD = D // 128        # 4 contraction chunks
G3 = (3 * D) // 128  # 12 gate chunks
JC = J // 128        # 5 output chunks
NTOK = T * BL        # 2056 (token, batch) pairs per core
BOS = 0

# Token blocks for bulk matmuls: blocks of 64 time steps (= 512 free cols).
TBLK = 64

_PROG_CACHE = {}
_RUNNER_CACHE = {}


def _build_program(t_steps, phases=("gi", "scan", "out")):
    import concourse.bass as bass
    import concourse.tile as tile
    from concourse import bacc, mybir

    f32 = mybir.dt.float32
    bf16 = mybir.dt.float16
    i32 = mybir.dt.int32
    ds = bass.ds
    AF = mybir.ActivationFunctionType

    ntok = t_steps * BL
    tblocks = [
        (i * TBLK, min(TBLK, t_steps - i * TBLK))
        for i in range((t_steps + TBLK - 1) // TBLK)
    ]

    nc = bacc.Bacc(
        "TRN2",
        target_bir_lowering=False,
        debug=False,
        enable_asserts=True,
        num_devices=1,
    )

    xt_d = nc.dram_tensor("xt", [128, KD, ntok], bf16, kind="ExternalInput").ap()
    wih_d = nc.dram_tensor("wih", [128, KD, G3, 128], bf16, kind="ExternalInput").ap()
    whh_d = nc.dram_tensor("whh", [128, KD, G3, 128], bf16, kind="ExternalInput").ap()
    lin_d = nc.dram_tensor("lin", [128, KD, JC, 128], bf16, kind="ExternalInput").ap()
    brz_d = nc.dram_tensor("brz", [128, 8], f32, kind="ExternalInput").ap()
    bni_d = nc.dram_tensor("bni", [128, KD], f32, kind="ExternalInput").ap()
    bhn_d = nc.dram_tensor("bhn", [128, KD, BL], f32, kind="ExternalInput").ap()
    ident_d = nc.dram_tensor("ident", [128, 128], bf16, kind="ExternalInput").ap()
    bhnb_d = nc.dram_tensor("bhnb", [128, KD, BL], bf16, kind="ExternalInput").ap()
    linb_d = nc.dram_tensor("linb", [128, JC], f32, kind="ExternalInput").ap()
    h0_d = nc.dram_tensor("h0", [128, KD, BL], bf16, kind="ExternalInput").ap()
    nrep_d = nc.dram_tensor("nrep", [1, 1], i32, kind="ExternalInput").ap()
    outp_d = nc.dram_tensor("outp", [128, JC, ntok], f32, kind="ExternalOutput").ap()

    with tile.TileContext(nc) as tc:
        with tc.tile_pool(name="const", bufs=1) as constp:
            # Load everything resident into SBUF.
            xt = constp.tile([128, KD, ntok], bf16)
            nc.sync.dma_start(xt[:], xt_d[:])
            wih = constp.tile([128, KD, G3, 128], bf16)
            nc.sync.dma_start(wih[:], wih_d[:])
            whh = constp.tile([128, KD, G3, 128], bf16)
            nc.sync.dma_start(whh[:], whh_d[:])
            lin = constp.tile([128, KD, JC, 128], bf16)
            nc.sync.dma_start(lin[:], lin_d[:])
            brz = constp.tile([128, 8], f32)
            nc.sync.dma_start(brz[:], brz_d[:])
            bni = constp.tile([128, KD], f32)
            nc.sync.dma_start(bni[:], bni_d[:])
            bhn = constp.tile([128, KD, BL], f32)
            nc.sync.dma_start(bhn[:], bhn_d[:])
            ident = constp.tile([128, 128], bf16)
            nc.sync.dma_start(ident[:], ident_d[:])
            bhnb = constp.tile([128, KD, BL], bf16)
            nc.sync.dma_start(bhnb[:], bhnb_d[:])
            linb = constp.tile([128, JC], f32)
            nc.sync.dma_start(linb[:], linb_d[:])
            h0 = constp.tile([128, KD, BL], bf16)
            nc.sync.dma_start(h0[:], h0_d[:])
            nrep_sb = constp.tile([1, 1], i32)
            nc.sync.dma_start(nrep_sb[:], nrep_d[:])

            # Big persistent buffers.
            git = constp.tile([128, G3, ntok], bf16)          # input-side gates
            hall = constp.tile([128, KD, t_steps, BL], bf16)  # hidden history
            oall = constp.tile([128, JC, ntok], f32)          # staged outputs

            def emit_phase1(gips):
                # Gi = x @ w_ih.T (+ biases), bulk, token-blocked.
                for g in range(G3):
                    bias_ap = brz[:, g : g + 1] if g < 8 else bni[:, g - 8 : g - 7]
                    for t0, tn in tblocks:
                        sz = tn * BL
                        ps = gips.tile([128, TBLK * BL], f32, tag="gi")
                        for k in range(KD):
                            nc.tensor.matmul(
                                ps[:, :sz],
                                wih[:, k, g, :],
                                xt[:, k, ds(t0 * BL, sz)],
                                start=(k == 0),
                                stop=(k == KD - 1),
                            )
                        nc.vector.tensor_scalar(
                            git[:, g, ds(t0 * BL, sz)],
                            ps[:, :sz],
                            bias_ap,
                            None,
                            mybir.AluOpType.add,
                        )

            def emit_scan_step(t, scanps, ew):
                # Short-chain step: the input-side gate values (git) and the
                # n-gate hidden bias are seeded into PSUM via identity
                # matmuls (start=True, BEFORE the h-side accumulation — a
                # start mid-group wipes sibling slices), so the elementwise
                # chain starts with sigmoid-from-PSUM directly (z negated via
                # scale=-1).
                h_prev = h0 if t == 0 else hall[:, :, t - 1, :]
                ps_r = scanps.tile([128, KD, BL], f32, tag="ps_r")
                ps_z = scanps.tile([128, KD, BL], f32, tag="ps_z")
                ps_n = scanps.tile([128, KD, BL], f32, tag="ps_n")
                # Gate order r, n, z: r's consumers start earliest; the z tail
                # (zc -> e -> h') is short.
                for gg, ps, fold_rhs in (
                    (0, ps_r, git[:, 0:KD, ds(t * BL, BL)]),
                    (2, ps_n, bhnb[:]),
                    (1, ps_z, git[:, KD : 2 * KD, ds(t * BL, BL)]),
                ):
                    nc.tensor.matmul(
                        ps[:], ident[:], fold_rhs,
                        start=True, stop=False, skip_group_check=True,
                    )
                    for j in range(KD):
                        g = gg * KD + j
                        for k in range(KD):
                            nc.tensor.matmul(
                                ps[:, j, :],
                                whh[:, k, g, :],
                                h_prev[:, k, :],
                                start=False,
                                stop=(j == KD - 1 and k == KD - 1),
                                skip_group_check=True,
                            )
                gi_n = git[:, 2 * KD : 3 * KD, ds(t * BL, BL)]

                r = ew.tile([128, KD, BL], f32, tag="r")
                nc.scalar.activation(r[:], ps_r[:], AF.Sigmoid)
                zc = ew.tile([128, KD, BL], f32, tag="zc")
                nc.scalar.activation(zc[:], ps_z[:], AF.Sigmoid, scale=-1.0)
                t1 = ew.tile([128, KD, BL], f32, tag="t1")
                nc.vector.tensor_mul(t1[:], ps_n[:], r[:])
                t2 = ew.tile([128, KD, BL], f32, tag="t2")
                nc.vector.tensor_add(t2[:], t1[:], gi_n)
                n_g = ew.tile([128, KD, BL], f32, tag="n_g")
                nc.scalar.activation(n_g[:], t2[:], AF.Tanh)
                d_g = ew.tile([128, KD, BL], f32, tag="d_g")
                nc.vector.tensor_sub(d_g[:], n_g[:], h_prev[:])
                e_g = ew.tile([128, KD, BL], f32, tag="e_g")
                nc.vector.tensor_mul(e_g[:], zc[:], d_g[:])
                nc.vector.tensor_add(hall[:, :, t, :], h_prev[:], e_g[:])

            def emit_scan_step_A(t, scanps, ew):
                # Shared psum tile, no folds, old-style ew adds.
                h_prev = h0 if t == 0 else hall[:, :, t - 1, :]
                ps = scanps.tile([128, G3, BL], f32, tag="ps")
                for gg in (0, 2, 1):
                    for j in range(KD):
                        g = gg * KD + j
                        for k in range(KD):
                            nc.tensor.matmul(
                                ps[:, g, :],
                                whh[:, k, g, :],
                                h_prev[:, k, :],
                                start=(k == 0),
                                stop=(k == KD - 1),
                                skip_group_check=True,
                            )
                ps_r = ps[:, 0:KD, :]
                ps_z = ps[:, KD : 2 * KD, :]
                ps_n = ps[:, 2 * KD : 3 * KD, :]
                gi_r = git[:, 0:KD, ds(t * BL, BL)]
                gi_z = git[:, KD : 2 * KD, ds(t * BL, BL)]
                gi_n = git[:, 2 * KD : 3 * KD, ds(t * BL, BL)]
                pre_r = ew.tile([128, KD, BL], f32, tag="pre_r")
                nc.vector.tensor_add(pre_r[:], ps_r, gi_r)
                r = ew.tile([128, KD, BL], f32, tag="r")
                nc.scalar.activation(r[:], pre_r[:], AF.Sigmoid)
                t1a = ew.tile([128, KD, BL], f32, tag="t1a")
                nc.vector.tensor_add(t1a[:], ps_n, bhn[:])
                t1 = ew.tile([128, KD, BL], f32, tag="t1")
                nc.vector.tensor_mul(t1[:], t1a[:], r[:])
                t2 = ew.tile([128, KD, BL], f32, tag="t2")
                nc.vector.tensor_add(t2[:], t1[:], gi_n)
                n_g = ew.tile([128, KD, BL], f32, tag="n_g")
                nc.scalar.activation(n_g[:], t2[:], AF.Tanh)
                d_g = ew.tile([128, KD, BL], f32, tag="d_g")
                nc.vector.tensor_sub(d_g[:], n_g[:], h_prev[:])
                pre_z = ew.tile([128, KD, BL], f32, tag="pre_z")
                nc.vector.scalar_tensor_tensor(
                    pre_z[:], ps_z, -1.0, gi_z,
                    mybir.AluOpType.mult, mybir.AluOpType.subtract,
                )
                zc = ew.tile([128, KD, BL], f32, tag="zc")
                nc.scalar.activation(zc[:], pre_z[:], AF.Sigmoid)
                e_g = ew.tile([128, KD, BL], f32, tag="e_g")
                nc.vector.tensor_mul(e_g[:], zc[:], d_g[:])
                nc.vector.tensor_add(hall[:, :, t, :], h_prev[:], e_g[:])

            def emit_scan_step_B(t, scanps, ew):
                # Separate psum tiles per gate + folds + short ew chain.
                h_prev = h0 if t == 0 else hall[:, :, t - 1, :]
                ps_r = scanps.tile([128, KD, BL], f32, tag="ps_r")
                ps_z = scanps.tile([128, KD, BL], f32, tag="ps_z")
                ps_n = scanps.tile([128, KD, BL], f32, tag="ps_n")
                for gg, ps, fold_rhs in (
                    (0, ps_r, git[:, 0:KD, ds(t * BL, BL)]),
                    (2, ps_n, bhnb[:]),
                    (1, ps_z, git[:, KD : 2 * KD, ds(t * BL, BL)]),
                ):
                    for j in range(KD):
                        g = gg * KD + j
                        for k in range(KD):
                            nc.tensor.matmul(
                                ps[:, j, :],
                                whh[:, k, g, :],
                                h_prev[:, k, :],
                                start=(k == 0),
                                stop=False,
                                skip_group_check=True,
                            )
                    nc.tensor.matmul(
                        ps[:],
                        ident[:],
                        fold_rhs,
                        start=False,
                        stop=True,
                        skip_group_check=True,
                    )
                gi_n = git[:, 2 * KD : 3 * KD, ds(t * BL, BL)]
                r = ew.tile([128, KD, BL], f32, tag="r")
                nc.scalar.activation(r[:], ps_r[:], AF.Sigmoid)
                zc = ew.tile([128, KD, BL], f32, tag="zc")
                nc.scalar.activation(zc[:], ps_z[:], AF.Sigmoid, scale=-1.0)
                t1 = ew.tile([128, KD, BL], f32, tag="t1")
                nc.vector.tensor_mul(t1[:], ps_n[:], r[:])
                t2 = ew.tile([128, KD, BL], f32, tag="t2")
                nc.vector.tensor_add(t2[:], t1[:], gi_n)
                n_g = ew.tile([128, KD, BL], f32, tag="n_g")
                nc.scalar.activation(n_g[:], t2[:], AF.Tanh)
                d_g = ew.tile([128, KD, BL], f32, tag="d_g")
                nc.vector.tensor_sub(d_g[:], n_g[:], h_prev[:])
                e_g = ew.tile([128, KD, BL], f32, tag="e_g")
                nc.vector.tensor_mul(e_g[:], zc[:], d_g[:])
                nc.vector.tensor_add(hall[:, :, t, :], h_prev[:], e_g[:])

            def emit_scan_step_C(t, scanps, ew):
                # Folds + old-style ew (with copies where adds were): isolates
                # fold-matmul correctness.
                h_prev = h0 if t == 0 else hall[:, :, t - 1, :]
                ps_r = scanps.tile([128, KD, BL], f32, tag="ps_r")
                ps_z = scanps.tile([128, KD, BL], f32, tag="ps_z")
                ps_n = scanps.tile([128, KD, BL], f32, tag="ps_n")
                for gg, ps, fold_rhs in (
                    (0, ps_r, git[:, 0:KD, ds(t * BL, BL)]),
                    (2, ps_n, bhnb[:]),
                    (1, ps_z, git[:, KD : 2 * KD, ds(t * BL, BL)]),
                ):
                    for j in range(KD):
                        g = gg * KD + j
                        for k in range(KD):
                            nc.tensor.matmul(
                                ps[:, j, :],
                                whh[:, k, g, :],
                                h_prev[:, k, :],
                                start=(k == 0),
                                stop=False,
                                skip_group_check=True,
                            )
                    nc.tensor.matmul(
                        ps[:],
                        ident[:],
                        fold_rhs,
                        start=False,
                        stop=True,
                        skip_group_check=True,
                    )
                gi_n = git[:, 2 * KD : 3 * KD, ds(t * BL, BL)]
                pre_r = ew.tile([128, KD, BL], f32, tag="pre_r")
                nc.vector.tensor_copy(pre_r[:], ps_r[:])
                r = ew.tile([128, KD, BL], f32, tag="r")
                nc.scalar.activation(r[:], pre_r[:], AF.Sigmoid)
                t1 = ew.tile([128, KD, BL], f32, tag="t1")
                nc.vector.tensor_mul(t1[:], ps_n[:], r[:])
                t2 = ew.tile([128, KD, BL], f32, tag="t2")
                nc.vector.tensor_add(t2[:], t1[:], gi_n)
                n_g = ew.tile([128, KD, BL], f32, tag="n_g")
                nc.scalar.activation(n_g[:], t2[:], AF.Tanh)
                d_g = ew.tile([128, KD, BL], f32, tag="d_g")
                nc.vector.tensor_sub(d_g[:], n_g[:], h_prev[:])
                pre_z = ew.tile([128, KD, BL], f32, tag="pre_z")
                nc.vector.tensor_scalar(
                    pre_z[:], ps_z[:], -1.0, None, mybir.AluOpType.mult,
                )
                zc = ew.tile([128, KD, BL], f32, tag="zc")
                nc.scalar.activation(zc[:], pre_z[:], AF.Sigmoid)
                e_g = ew.tile([128, KD, BL], f32, tag="e_g")
                nc.vector.tensor_mul(e_g[:], zc[:], d_g[:])
                nc.vector.tensor_add(hall[:, :, t, :], h_prev[:], e_g[:])

            def emit_scan_step_D(t, scanps, ew):
                # No folds; DVE adds as in old; but sigmoid(scale=-1) for z
                # and sigmoid straight from an SBUF pre-act: isolates the
                # activation-scale path.
                h_prev = h0 if t == 0 else hall[:, :, t - 1, :]
                ps_r = scanps.tile([128, KD, BL], f32, tag="ps_r")
                ps_z = scanps.tile([128, KD, BL], f32, tag="ps_z")
                ps_n = scanps.tile([128, KD, BL], f32, tag="ps_n")
                for gg, ps in ((0, ps_r), (2, ps_n), (1, ps_z)):
                    for j in range(KD):
                        g = gg * KD + j
                        for k in range(KD):
                            nc.tensor.matmul(
                                ps[:, j, :],
                                whh[:, k, g, :],
                                h_prev[:, k, :],
                                start=(k == 0),
                                stop=(k == KD - 1),
                            )
                gi_r = git[:, 0:KD, ds(t * BL, BL)]
                gi_z = git[:, KD : 2 * KD, ds(t * BL, BL)]
                gi_n = git[:, 2 * KD : 3 * KD, ds(t * BL, BL)]
                pre_r = ew.tile([128, KD, BL], f32, tag="pre_r")
                nc.vector.tensor_add(pre_r[:], ps_r[:], gi_r)
                r = ew.tile([128, KD, BL], f32, tag="r")
                nc.scalar.activation(r[:], pre_r[:], AF.Sigmoid)
                t1a = ew.tile([128, KD, BL], f32, tag="t1a")
                nc.vector.tensor_add(t1a[:], ps_n[:], bhn[:])
                t1 = ew.tile([128, KD, BL], f32, tag="t1")
                nc.vector.tensor_mul(t1[:], t1a[:], r[:])
                t2 = ew.tile([128, KD, BL], f32, tag="t2")
                nc.vector.tensor_add(t2[:], t1[:], gi_n)
                n_g = ew.tile([128, KD, BL], f32, tag="n_g")
                nc.scalar.activation(n_g[:], t2[:], AF.Tanh)
                d_g = ew.tile([128, KD, BL], f32, tag="d_g")
                nc.vector.tensor_sub(d_g[:], n_g[:], h_prev[:])
                pre_z2 = ew.tile([128, KD, BL], f32, tag="pre_z2")
                nc.vector.tensor_add(pre_z2[:], ps_z[:], gi_z)
                zc = ew.tile([128, KD, BL], f32, tag="zc")
                nc.scalar.activation(zc[:], pre_z2[:], AF.Sigmoid, scale=-1.0)
                e_g = ew.tile([128, KD, BL], f32, tag="e_g")
                nc.vector.tensor_mul(e_g[:], zc[:], d_g[:])
                nc.vector.tensor_add(hall[:, :, t, :], h_prev[:], e_g[:])

            def emit_scan_step_old(t, scanps, ew):
                h_prev = h0 if t == 0 else hall[:, :, t - 1, :]
                ps_r = scanps.tile([128, KD, BL], f32, tag="ps_r")
                ps_z = scanps.tile([128, KD, BL], f32, tag="ps_z")
                ps_n = scanps.tile([128, KD, BL], f32, tag="ps_n")
                for gg, ps in ((0, ps_r), (2, ps_n), (1, ps_z)):
                    for j in range(KD):
                        g = gg * KD + j
                        for k in range(KD):
                            nc.tensor.matmul(
                                ps[:, j, :],
                                whh[:, k, g, :],
                                h_prev[:, k, :],
                                start=(k == 0),
                                stop=(k == KD - 1),
                            )
                gi_r = git[:, 0:KD, ds(t * BL, BL)]
                gi_z = git[:, KD : 2 * KD, ds(t * BL, BL)]
                gi_n = git[:, 2 * KD : 3 * KD, ds(t * BL, BL)]

                pre_r = ew.tile([128, KD, BL], f32, tag="pre_r")
                nc.vector.tensor_add(pre_r[:], ps_r[:], gi_r)
                r = ew.tile([128, KD, BL], f32, tag="r")
                nc.scalar.activation(r[:], pre_r[:], AF.Sigmoid)

                t1a = ew.tile([128, KD, BL], f32, tag="t1a")
                nc.vector.tensor_add(t1a[:], ps_n[:], bhn[:])
                t1 = ew.tile([128, KD, BL], f32, tag="t1")
                nc.vector.tensor_mul(t1[:], t1a[:], r[:])
                t2 = ew.tile([128, KD, BL], f32, tag="t2")
                nc.vector.tensor_add(t2[:], t1[:], gi_n)
                n_g = ew.tile([128, KD, BL], f32, tag="n_g")
                nc.scalar.activation(n_g[:], t2[:], AF.Tanh)
                d_g = ew.tile([128, KD, BL], f32, tag="d_g")
                nc.vector.tensor_sub(d_g[:], n_g[:], h_prev[:])

                pre_z = ew.tile([128, KD, BL], f32, tag="pre_z")
                nc.vector.scalar_tensor_tensor(
                    pre_z[:], ps_z[:], -1.0, gi_z,
                    mybir.AluOpType.mult, mybir.AluOpType.subtract,
                )
                zc = ew.tile([128, KD, BL], f32, tag="zc")
                nc.scalar.activation(zc[:], pre_z[:], AF.Sigmoid)

                e_g = ew.tile([128, KD, BL], f32, tag="e_g")
                nc.vector.tensor_mul(e_g[:], zc[:], d_g[:])
                nc.vector.tensor_add(hall[:, :, t, :], h_prev[:], e_g[:])

            def emit_phase3(ops):
                # out = h @ lin_w.T + lin_b, bulk, staged into SBUF.
                for c in range(JC):
                    for t0, tn in tblocks:
                        sz = tn * BL
                        ps = ops.tile([128, TBLK * BL], f32, tag="op")
                        for k in range(KD):
                            nc.tensor.matmul(
                                ps[:, :sz],
                                lin[:, k, c, :],
                                hall[:, k, ds(t0, tn), :],
                                start=(k == 0),
                                stop=(k == KD - 1),
                            )
                        nc.vector.tensor_scalar(
                            oall[:, c, ds(t0 * BL, sz)],
                            ps[:, :sz],
                            linb[:, c : c + 1],
                            None,
                            mybir.AluOpType.add,
                        )

            def emit_scan_pe48(t, scanps, sinkp):
                # Probe: PE throughput of the thin D-major matmul pattern
                # (no elementwise, h held constant).
                ps_r = scanps.tile([128, KD, BL], f32, tag="ps_r")
                ps_z = scanps.tile([128, KD, BL], f32, tag="ps_z")
                ps_n = scanps.tile([128, KD, BL], f32, tag="ps_n")
                for gg, ps in ((0, ps_r), (2, ps_n), (1, ps_z)):
                    for j in range(KD):
                        g = gg * KD + j
                        for k in range(KD):
                            nc.tensor.matmul(
                                ps[:, j, :],
                                whh[:, k, g, :],
                                h0[:, k, :],
                                start=(k == 0),
                                stop=(k == KD - 1),
                            )
                for nm, ps in (("skr", ps_r), ("skz", ps_z), ("skn", ps_n)):
                    sink = sinkp.tile([128, KD, BL], f32, tag=nm)
                    nc.scalar.copy(sink[:], ps[:])

            def emit_scan_ew(t, ew):
                # Probe: the elementwise chain alone (psum inputs faked with
                # git slices), feedback through hall kept.
                h_prev = h0 if t == 0 else hall[:, :, t - 1, :]
                gi_r = git[:, 0:KD, ds(t * BL, BL)]
                gi_z = git[:, KD : 2 * KD, ds(t * BL, BL)]
                gi_n = git[:, 2 * KD : 3 * KD, ds(t * BL, BL)]
                pre_r = ew.tile([128, KD, BL], f32, tag="pre_r")
                nc.vector.tensor_add(pre_r[:], gi_z, gi_r)
                r = ew.tile([128, KD, BL], f32, tag="r")
                nc.scalar.activation(r[:], pre_r[:], AF.Sigmoid)
                t1a = ew.tile([128, KD, BL], f32, tag="t1a")
                nc.vector.tensor_add(t1a[:], gi_n, bhn[:])
                t1 = ew.tile([128, KD, BL], f32, tag="t1")
                nc.vector.tensor_mul(t1[:], t1a[:], r[:])
                t2 = ew.tile([128, KD, BL], f32, tag="t2")
                nc.vector.tensor_add(t2[:], t1[:], gi_n)
                n_g = ew.tile([128, KD, BL], f32, tag="n_g")
                nc.scalar.activation(n_g[:], t2[:], AF.Tanh)
                d_g = ew.tile([128, KD, BL], f32, tag="d_g")
                nc.vector.tensor_sub(d_g[:], n_g[:], h_prev[:])
                pre_z = ew.tile([128, KD, BL], f32, tag="pre_z")
                nc.vector.scalar_tensor_tensor(
                    pre_z[:], gi_z, -1.0, gi_r,
                    mybir.AluOpType.mult, mybir.AluOpType.subtract,
                )
                zc = ew.tile([128, KD, BL], f32, tag="zc")
                nc.scalar.activation(zc[:], pre_z[:], AF.Sigmoid)
                e_g = ew.tile([128, KD, BL], f32, tag="e_g")
                nc.vector.tensor_mul(e_g[:], zc[:], d_g[:])
                nc.vector.tensor_add(hall[:, :, t, :], h_prev[:], e_g[:])

            # Probe variants: init tiles whose producer phase is absent so the
            # Tile framework sees every read tile written.
            if "gi" not in phases:
                nc.scalar.memzero(git[:])
            if "scan" not in phases:
                nc.scalar.memzero(hall[:])
            if "out" not in phases:
                nc.scalar.memzero(oall[:])

            nrep = nc.values_load(
                nrep_sb[0:1, 0:1], min_val=1, max_val=1 << 20,
                skip_runtime_bounds_check=True,
            )
            with tc.For_i(0, nrep) as _i:
                if "gi" in phases:
                    with tc.tile_pool(name="gips", bufs=2, space="PSUM") as gips:
                        emit_phase1(gips)
                if "scan" in phases:
                    with tc.tile_pool(name="scanps", bufs=2, space="PSUM") as scanps, \
                         tc.tile_pool(name="ew", bufs=3) as ew:
                        for t in range(t_steps):
                            emit_scan_step(t, scanps, ew)
                if "scanold" in phases:
                    with tc.tile_pool(name="scanps", bufs=2, space="PSUM") as scanps, \
                         tc.tile_pool(name="ew", bufs=3) as ew:
                        for t in range(t_steps):
                            emit_scan_step_old(t, scanps, ew)
                if "scanA" in phases:
                    with tc.tile_pool(name="scanps", bufs=4, space="PSUM") as scanps, \
                         tc.tile_pool(name="ew", bufs=3) as ew:
                        for t in range(t_steps):
                            emit_scan_step_A(t, scanps, ew)
                if "scanB" in phases:
                    with tc.tile_pool(name="scanps", bufs=2, space="PSUM") as scanps, \
                         tc.tile_pool(name="ew", bufs=3) as ew:
                        for t in range(t_steps):
                            emit_scan_step_B(t, scanps, ew)
                if "scanC" in phases:
                    with tc.tile_pool(name="scanps", bufs=2, space="PSUM") as scanps, \
                         tc.tile_pool(name="ew", bufs=3) as ew:
                        for t in range(t_steps):
                            emit_scan_step_C(t, scanps, ew)
                if "scanD" in phases:
                    with tc.tile_pool(name="scanps", bufs=2, space="PSUM") as scanps, \
                         tc.tile_pool(name="ew", bufs=3) as ew:
                        for t in range(t_steps):
                            emit_scan_step_D(t, scanps, ew)
                if "pe48" in phases:
                    with tc.tile_pool(name="scanps", bufs=2, space="PSUM") as scanps, \
                         tc.tile_pool(name="sink", bufs=2) as sinkp:
                        for t in range(t_steps):
                            emit_scan_pe48(t, scanps, sinkp)
                if "ewonly" in phases:
                    with tc.tile_pool(name="ew", bufs=3) as ew:
                        for t in range(t_steps):
                            emit_scan_ew(t, ew)
                if "out" in phases:
                    with tc.tile_pool(name="ops", bufs=2, space="PSUM") as ops:
                        emit_phase3(ops)

            # One bulk DMA of the staged outputs.
            nc.sync.dma_start(outp_d[:], oall[:])

    nc.compile()
    return nc


def _get_program(t_steps, phases=("gi", "scan", "out")):
    key = (t_steps, tuple(phases))
    if key not in _PROG_CACHE:
        _PROG_CACHE[key] = _build_program(t_steps, phases)
    return _PROG_CACHE[key]


def _make_runner(nc, n_cores):
    """Build a reusable jitted SPMD runner for a compiled Bass program.

    Mirrors concourse.bass_utils.run_bass_kernel_spmd's axon path
    (bass2jax.run_bass_via_pjrt), but constructs the jitted callable once so
    repeated kernel() calls don't re-trace/re-compile host-side.
    """
    import jax
    from jax.sharding import Mesh, PartitionSpec
    from jax.experimental.shard_map import shard_map
    from concourse import mybir
    from concourse.bass2jax import (
        _bass_exec_p,
        install_neuronx_cc_hook,
        partition_id_tensor,
    )

    install_neuronx_cc_hook()

    partition_name = nc.partition_id_tensor.name if nc.partition_id_tensor else None
    in_names, out_names, out_avals = [], [], []
    for alloc in nc.m.functions[0].allocations:
        if not isinstance(alloc, mybir.MemoryLocationSet):
            continue
        name = alloc.memorylocations[0].name
        if alloc.kind == "ExternalInput":
            if name != partition_name:
                in_names.append(name)
        elif alloc.kind == "ExternalOutput":
            out_names.append(name)
            shape = tuple(alloc.tensor_shape)
            dtype = mybir.dt.np(alloc.dtype)
            out_avals.append(jax.core.ShapedArray(shape, dtype))
    n_params = len(in_names)
    n_outs = len(out_avals)
    all_names = in_names + out_names + ([partition_name] if partition_name else [])

    def _body(*args):
        operands = list(args)
        if partition_name is not None:
            operands.append(partition_id_tensor())
        outs = _bass_exec_p.bind(
            *operands,
            out_avals=tuple(out_avals),
            in_names=tuple(all_names),
            out_names=tuple(out_names),
            lowering_input_output_aliases=(),
            sim_require_finite=True,
            sim_require_nnan=True,
            nc=nc,
        )
        return tuple(outs)

    import jax as _jax
    devices = _jax.devices()[:n_cores]
    assert len(devices) == n_cores
    mesh = Mesh(np.asarray(devices), ("core",))
    in_specs = (PartitionSpec("core"),) * (n_params + n_outs)
    out_specs = (PartitionSpec("core"),) * n_outs
    donate = tuple(range(n_params, n_params + n_outs))
    sharded = _jax.jit(
        shard_map(_body, mesh=mesh, in_specs=in_specs, out_specs=out_specs,
                  check_rep=False),
        donate_argnums=donate, keep_unused=True,
    )

    def run(in_maps):
        per_core = [[np.asarray(m[name]) for name in in_names] for m in in_maps]
        concat_in = [
            np.concatenate([per_core[c][i] for c in range(n_cores)], axis=0)
            for i in range(n_params)
        ]
        concat_zeros = [
            np.zeros((n_cores * a.shape[0], *a.shape[1:]), a.dtype)
            for a in out_avals
        ]
        out_arrs = sharded(*concat_in, *concat_zeros)
        return [
            {
                name: np.asarray(out_arrs[i]).reshape(
                    n_cores, *out_avals[i].shape)[c]
                for i, name in enumerate(out_names)
            }
            for c in range(n_cores)
        ]

    return run


def _get_runner(t_steps, phases=("gi", "scan", "out")):
    key = (t_steps, tuple(phases))
    if key not in _RUNNER_CACHE:
        nc = _get_program(t_steps, phases)
        _RUNNER_CACHE[key] = _make_runner(nc, NCORES)
    return _RUNNER_CACHE[key]


def kernel(src_tokens, src_lengths, embed_w, w_ih, w_hh, b_ih, b_hh, lin_w, lin_b,
           init_state, _t_steps=T, _nrep=1, _phases=("gi", "scan", "out"),
           **_ignored):
    src_tokens = np.asarray(src_tokens)
    embed_w = np.asarray(embed_w, dtype=np.float32)
    w_ih = np.asarray(w_ih, dtype=np.float32)
    w_hh = np.asarray(w_hh, dtype=np.float32)
    b_ih = np.asarray(b_ih, dtype=np.float32)
    b_hh = np.asarray(b_hh, dtype=np.float32)
    lin_w = np.asarray(lin_w, dtype=np.float32)
    lin_b = np.asarray(lin_b, dtype=np.float32)
    init_state = np.asarray(init_state, dtype=np.float32)

    t_steps = _t_steps
    ntok = t_steps * BL

    # Host prep: embedding gather + layout shuffles (no FLOPs).
    tokens = np.concatenate(
        [np.full((B, 1), BOS, dtype=src_tokens.dtype), src_tokens], axis=1
    )[:, :t_steps]                                   # [B, T]
    X = embed_w[tokens].astype(BF16)                 # [B, T, D]

    def dmaj(vec):  # [D] -> [128, KD]
        return np.ascontiguousarray(vec.reshape(KD, 128).T)

    wih_t = np.ascontiguousarray(
        w_ih.reshape(G3, 128, KD, 128).transpose(3, 2, 0, 1)).astype(BF16)
    whh_t = np.ascontiguousarray(
        w_hh.reshape(G3, 128, KD, 128).transpose(3, 2, 0, 1)).astype(BF16)
    lin_t = np.ascontiguousarray(
        lin_w.reshape(JC, 128, KD, 128).transpose(3, 2, 0, 1)).astype(BF16)
    brz = np.ascontiguousarray((b_ih + b_hh)[: 2 * D].reshape(8, 128).T)
    bni = dmaj(b_ih[2 * D :])
    bhn = np.ascontiguousarray(
        np.broadcast_to(dmaj(b_hh[2 * D :])[:, :, None], (128, KD, BL)))
    linb = np.ascontiguousarray(lin_b.reshape(JC, 128).T)
    h0 = np.ascontiguousarray(
        np.broadcast_to(dmaj(init_state)[:, :, None], (128, KD, BL))).astype(BF16)

    shared = {
        "wih": wih_t, "whh": whh_t, "lin": lin_t,
        "brz": brz.astype(np.float32), "bni": bni.astype(np.float32),
        "bhn": bhn.astype(np.float32), "linb": linb.astype(np.float32),
        "h0": h0, "nrep": np.full((1, 1), _nrep, dtype=np.int32),
        "ident": np.eye(128, dtype=BF16),
        "bhnb": bhn.astype(BF16),
    }
    in_maps = []
    for c in range(NCORES):
        xc = X[c * BL : (c + 1) * BL]                # [BL, T, D]
        xt = np.ascontiguousarray(
            xc.reshape(BL, t_steps, KD, 128).transpose(3, 2, 1, 0)
        ).reshape(128, KD, ntok)
        in_maps.append({**shared, "xt": np.ascontiguousarray(xt)})

    run = _get_runner(t_steps, _phases)
    results = run(in_maps)

    out = np.empty((B, t_steps, J), dtype=np.float32)
    for c in range(NCORES):
        o = results[c]["outp"]                       # [128, JC, ntok]
        o = o.reshape(128, JC, t_steps, BL).transpose(3, 2, 1, 0)  # [BL,T,JC,128]
        out[c * BL : (c + 1) * BL] = o.reshape(BL, t_steps, J)
    return out


if __name__ == "__main__":
    # Quick smoke test with a tiny number of steps.
    t_steps = int(os.environ.get("KERNEL_T", "8"))
    nrep = int(os.environ.get("KERNEL_NREP", "1"))
    rng = np.random.default_rng(0)
    ins = {
        "src_tokens": rng.integers(0, V, size=(B, U)).astype(np.int64),
        "src_lengths": rng.integers(1, U, size=(B,)).astype(np.int32),
        "embed_w": (rng.standard_normal((V, D)) * 0.02).astype(np.float32),
        "w_ih": (rng.standard_normal((3 * D, D)) / np.sqrt(D)).astype(np.float32),
        "w_hh": (rng.standard_normal((3 * D, D)) / np.sqrt(D)).astype(np.float32),
        "b_ih": (rng.standard_normal(3 * D) * 0.01).astype(np.float32),
        "b_hh": (rng.standard_normal(3 * D) * 0.01).astype(np.float32),
        "lin_w": (rng.standard_normal((J, D)) / np.sqrt(D)).astype(np.float32),
        "lin_b": (rng.standard_normal(J) * 0.01).astype(np.float32),
        "init_state": rng.standard_normal(D).astype(np.float32),
    }
    actual = kernel(**ins, _t_steps=t_steps, _nrep=nrep)

    # numpy reference for t_steps
    tokens = np.concatenate(
        [np.zeros((B, 1), dtype=np.int64), ins["src_tokens"]], axis=1)[:, :t_steps]
    x_all = ins["embed_w"][tokens]
    h = np.broadcast_to(ins["init_state"], (B, D)).astype(np.float32)
    outs = []
    for t in range(t_steps):
        gi = x_all[:, t] @ ins["w_ih"].T + ins["b_ih"]
        gh = h @ ins["w_hh"].T + ins["b_hh"]
        i_r, i_z, i_n = np.split(gi, 3, axis=-1)
        h_r, h_z, h_n = np.split(gh, 3, axis=-1)
        r = 1 / (1 + np.exp(-(i_r + h_r)))
        z = 1 / (1 + np.exp(-(i_z + h_z)))
        n = np.tanh(i_n + r * h_n)
        h = (1 - z) * n + z * h
        outs.append(h @ ins["lin_w"].T + ins["lin_b"])
    expected = np.stack(outs, axis=1)
    err = np.abs(actual - expected)
    rel = np.linalg.norm(actual - expected) / np.linalg.norm(expected)
    print("max abs err:", err.max(), "rel l2:", rel)
